# revision 13
# baseline (speedup 1.0000x reference)
"""Trainium2 Bass kernel for CrossAttention (B=4, L=S=2048, DIM=1024, H=16, hd=64).

Sharding: data-parallel over (batch, L-half): core c handles batch c//2,
query rows [(c%2)*1024, (c%2+1)*1024).  Each core computes the QKV
projections for its slice (K/V duplicated within a batch pair), per-head
RMSNorm, masked softmax attention, and the output projection.

Device layout is feature-major ("transposed"): activations live as
[dim, tokens] so every matmul contraction dim is on SBUF partitions with
no on-device transposes.  The host pre-transposes q/kv and casts to bf16.

Softmax: after RMS norm |score| <= 8, so no running max is needed.  The
k-norm rsqrt/8 is folded into kh right after the K projection (rank-2 PE
broadcast via the head-indicator matmul), and the padding mask is folded
into V by zeroing masked va rows (masked exps are finite but contribute
nothing, including to the denominator's ones-column) -- the
attention-phase exp is therefore a BARE activation with no scale/bias AP
reads (~850ns vs ~1400ns per [128,1024] tile).  The denominator comes
from a 65th "ones" column appended to V; the division is deferred: o^T
is stored unnormalized, den rows are collected per head, one fast
approximate reciprocal runs at the end, and a rank-1 bf16 PE broadcast
normalizes o^T interleaved with the output projection per 512-col half.

Perf notes: the Sync engine costs ~650ns per dma_start trigger, so every
multi-tile transfer is batched into ONE dma_start over a big tile (wk,
kvt, wv, wq, qt, wo, ind, ind2 inputs; kh/va gather blobs in
partition-major layout; readbacks).  K-proj operands (wk, kvT) are
issued first so the PE starts ~13us in; wo is fetched during attention
into the recycled wk slot.  The attention loop runs ONE head at a time;
pv matmuls are emitted TWO chunks late so they never wait on the exp:
per 128-pos chunk the PE owes 4 N=512 matmuls (~0.95us) vs one
[128,1024] exp on ACT (~1.1us), so ACT paces the phase.  PSUM exactly
fits: 2x[128,1024] score tiles + 2x[128,1024] pv/aux tiles = 8 banks.
Rank-1 broadcast matmuls (q-norm, k-norm, o-norm) use bf16 operands
(full rate) instead of 4x-slow fp32.
"""

import sys

if "/opt/trn_rl_repo" not in sys.path:
    sys.path.insert(0, "/opt/trn_rl_repo")

from collections import deque

import numpy as np
import ml_dtypes

import concourse.bass as bass
import concourse.bacc as bacc
import concourse.tile as tile
from concourse import mybir
from concourse.bass_utils import run_bass_kernel_spmd

BF16 = ml_dtypes.bfloat16

B, L, S, DIM = 4, 2048, 2048, 1024
H, HD = 16, 64
N_CORES = 8
LC = L // 2          # query rows per core
KC = DIM // 128      # 128-partition chunks of DIM
EPS = 1e-5

TRACE = False        # set by test.py for profiling
LAST_RESULT = {}     # exec_time_ns etc. for test.py

_CACHE = {}


def _build(n_sc):
    """Build the SPMD Bass program; n_sc = number of 128-wide kv chunks."""
    fp32 = mybir.dt.float32
    bf16 = mybir.dt.bfloat16
    AF = mybir.ActivationFunctionType

    nc = bacc.Bacc("TRN2", target_bir_lowering=False, debug=False,
                   num_devices=N_CORES)

    qT_d = nc.dram_tensor("qT", [128, KC * LC], bf16, kind="ExternalInput")
    n_half = (n_sc + 1) // 2         # kv chunks computed locally per core
    W = n_half * 128                 # local kv width
    kvT_d = nc.dram_tensor("kvT", [128, KC * W], bf16, kind="ExternalInput")
    wq_d = nc.dram_tensor("wq", [128, KC * DIM], bf16, kind="ExternalInput")
    wk_d = nc.dram_tensor("wk", [128, KC * DIM], bf16, kind="ExternalInput")
    wv_d = nc.dram_tensor("wv", [128, KC * DIM], bf16, kind="ExternalInput")
    wo_d = nc.dram_tensor("wo", [128, KC * DIM], bf16, kind="ExternalInput")
    mask01_d = nc.dram_tensor("mask01", [128, 8], fp32, kind="ExternalInput")
    qw_d = nc.dram_tensor("qw", [128, 1], fp32, kind="ExternalInput")
    kw_d = nc.dram_tensor("kw", [128, 1], fp32, kind="ExternalInput")
    ind_d = nc.dram_tensor("ind", [128, KC * 16], bf16, kind="ExternalInput")
    ind2_d = nc.dram_tensor("ind2", [16, KC * 128], bf16,
                            kind="ExternalInput")
    out_d = nc.dram_tensor("out", [LC, DIM], fp32, kind="ExternalOutput")

    n_hg = (W + 511) // 512          # 512-wide groups over the local half
    VA_W = H * 65                    # 1040 va columns per chunk

    with tile.TileContext(nc) as tc:
        with (
            tc.tile_pool(name="wp", bufs=3) as wp,               # wk wv wq (+wo)
            tc.tile_pool(name="qtp", bufs=1) as qtp,             # qT, later oT
            tc.tile_pool(name="kvp", bufs=1) as kvp,             # kvT (+sq/exp)
            tc.tile_pool(name="qhp", bufs=KC) as qhp,            # qhT
            tc.tile_pool(name="khp", bufs=1) as khp,             # khT
            tc.tile_pool(name="vp", bufs=1) as vp,               # vh_aug
            tc.tile_pool(name="sp", bufs=1) as sp,               # constants
            tc.tile_pool(name="tp", bufs=4) as tp,               # f32 temps
            tc.tile_pool(name="dp", bufs=1, space="DRAM") as dp,     # blobs
            tc.tile_pool(name="pa", bufs=2, space="PSUM") as pa,     # proj/score
            tc.tile_pool(name="po", bufs=2, space="PSUM") as po,     # pv/aux
        ):
            # ---- inputs, one dma_start each; K-proj operands first ----
            wk_big = wp.tile([128, KC * DIM], bf16, name="wk", tag="wbig")
            nc.sync.dma_start(out=wk_big, in_=wk_d[:, :])
            kvt_big = kvp.tile([128, KC * W], bf16, name="kvt", tag="kv")
            nc.sync.dma_start(out=kvt_big, in_=kvT_d[:, :])
            ind_big = sp.tile([128, KC * 16], bf16, name="ind")
            nc.sync.dma_start(out=ind_big, in_=ind_d[:, :])
            kw_sb = sp.tile([128, 1], fp32, name="kw")
            nc.sync.dma_start(out=kw_sb, in_=kw_d[:, :])
            ind2_big = sp.tile([16, KC * 128], bf16, name="ind2")
            nc.sync.dma_start(out=ind2_big, in_=ind2_d[:, :])
            mask01_sb = sp.tile([128, 8], fp32, name="mask01")
            nc.sync.dma_start(out=mask01_sb, in_=mask01_d[:, :])
            qw_sb = sp.tile([128, 1], fp32, name="qw")
            nc.sync.dma_start(out=qw_sb, in_=qw_d[:, :])
            wv_big = wp.tile([128, KC * DIM], bf16, name="wv", tag="wbig")
            nc.sync.dma_start(out=wv_big, in_=wv_d[:, :])
            wq_big = wp.tile([128, KC * DIM], bf16, name="wq", tag="wbig")
            nc.sync.dma_start(out=wq_big, in_=wq_d[:, :])
            qt_big = qtp.tile([128, KC * LC], bf16, name="qt", tag="qt")
            nc.sync.dma_start(out=qt_big, in_=qT_d[:, :])
            # warmup: pay the one-time CC collective-launch cost (~12us)
            # during the input DMA phase with a tiny dependency-free gather
            wu_in = dp.tile([128], bf16, name="wu_in")
            wu_out = dp.tile([256], bf16, name="wu_out")
            nc.gpsimd.collective_compute(
                "AllGather", mybir.AluOpType.bypass,
                replica_groups=[[2 * x, 2 * x + 1]
                                for x in range(N_CORES // 2)],
                ins=[wu_in.opt()], outs=[wu_out.opt()])

            def wkc(k):
                return wk_big[:, k * DIM:(k + 1) * DIM]

            def wvc(k):
                return wv_big[:, k * DIM:(k + 1) * DIM]

            def wqc(k):
                return wq_big[:, k * DIM:(k + 1) * DIM]

            def qtc(k):
                return qt_big[:, k * LC:(k + 1) * LC]

            def kvtc(k):
                return kvt_big[:, k * W:(k + 1) * W]

            def indc(k):
                return ind_big[:, k * 16:(k + 1) * 16]

            def ind2c(k):
                return ind2_big[:, k * 128:(k + 1) * 128]

            epsq_sb = sp.tile([16, 1], fp32, name="epsq")
            nc.vector.memset(epsq_sb, EPS)
            epsk_sb = sp.tile([16, 1], fp32, name="epsk")
            nc.vector.memset(epsk_sb, 64.0 * EPS)

            # -------- K projection + skT on the LOCAL kv half --------
            khh_big = khp.tile([128, KC * W], bf16, name="khh", tag="khh")
            skrec_sb = [tp.tile([16, 512], bf16, name=f"skrec{sg}",
                                tag="small16b", bufs=2)
                        for sg in range(n_hg)]
            for sg in range(n_hg):
                wdt = min(512, W - sg * 512)
                ssk = po.tile([16, 512], fp32, name="ssk", tag="pv")
                pend = None
                for m in range(KC):
                    ps = pa.tile([128, 1024], fp32, name="proj_ps", tag="pa")
                    for k in range(KC):
                        nc.tensor.matmul(
                            ps[:, :wdt],
                            lhsT=wkc(k)[:, m * 128:(m + 1) * 128],
                            rhs=kvtc(k)[:, sg * 512:sg * 512 + wdt],
                            start=(k == 0), stop=(k == KC - 1))
                    if pend is not None:
                        pm, pq = pend
                        nc.tensor.matmul(
                            ssk[:, :wdt], lhsT=indc(pm), rhs=pq[:, :wdt],
                            start=(pm == 0), stop=False)
                    nc.vector.tensor_scalar_mul(
                        khh_big[:, m * W + sg * 512:m * W + sg * 512 + wdt],
                        ps[:, :wdt], kw_sb)
                    ksq = kvp.tile([128, 1024], bf16, name="sqt", tag="sq",
                                   bufs=6)
                    nc.scalar.activation(ksq[:, :wdt], ps[:, :wdt], AF.Square)
                    pend = (m, ksq)
                pm, pq = pend
                nc.tensor.matmul(ssk[:, :wdt], lhsT=indc(pm),
                                 rhs=pq[:, :wdt], start=False, stop=True)
                # 8*sqrt(mean+eps) = sqrt(sumsq + 64 eps); recip -> sk/8
                skr = tp.tile([16, 512], fp32, name="skr", tag="small16",
                              bufs=2)
                nc.scalar.activation(skr[:, :wdt], ssk[:, :wdt], AF.Sqrt,
                                     scale=1.0, bias=epsk_sb)
                skt = tp.tile([16, 512], fp32, name="skt", tag="small16c",
                              bufs=2)
                nc.vector.reciprocal_approx_fast(out=skt[:, :wdt],
                                                 in_=skr[:, :wdt])
                nc.vector.tensor_copy(skrec_sb[sg][:, :wdt], skt[:, :wdt])

            # -------- scale kh by sk/8, then AllGather 1 (overlaps V) ----
            TOTA = KC * 128 * W
            TOTB = n_half * 128 * VA_W
            groups = [[2 * x, 2 * x + 1] for x in range(N_CORES // 2)]
            blobA_loc = dp.tile([TOTA], bf16, name="blobA_loc")
            blobA_g = dp.tile([2 * TOTA], bf16, name="blobA_g")
            for m in range(KC):
                bcsk = po.tile([128, W], fp32, name="bcsk", tag="pv")
                for sg in range(n_hg):
                    wdt = min(512, W - sg * 512)
                    nc.tensor.matmul(
                        bcsk[:, sg * 512:sg * 512 + wdt],
                        lhsT=ind2c(m), rhs=skrec_sb[sg][:, :wdt],
                        start=True, stop=True)
                nc.vector.tensor_mul(khh_big[:, m * W:(m + 1) * W],
                                     khh_big[:, m * W:(m + 1) * W], bcsk)
            # p-major blob: (p, m, x)
            nc.sync.dma_start(
                out=blobA_loc.rearrange("(p x) -> p x", p=128),
                in_=khh_big[:, :])
            nc.gpsimd.collective_compute(
                "AllGather", mybir.AluOpType.bypass, replica_groups=groups,
                ins=[blobA_loc.opt()], outs=[blobA_g.opt()])

            # -------- V projection on the LOCAL kv half (ones-augmented) ----
            val_big = vp.tile([128, n_half * VA_W], bf16, name="val",
                              tag="val")
            for i in range(n_half):
                va = val_big[:, i * VA_W:(i + 1) * VA_W]
                ones_cols = bass.AP(tensor=va.tensor, offset=va.offset + 64,
                                    ap=[list(va.ap[0]), [65, H], [1, 1]])
                nc.vector.memset(ones_cols, 1.0)
                ps = pa.tile([128, 1024], fp32, name="proj_ps", tag="pa")
                for k in range(KC):          # k outer: one LDW serves both jn
                    for jn in range(2):
                        nc.tensor.matmul(
                            ps[:, jn * 512:(jn + 1) * 512],
                            lhsT=kvtc(k)[:, i * 128:(i + 1) * 128],
                            rhs=wvc(k)[:, jn * 512:(jn + 1) * 512],
                            start=(k == 0), stop=(k == KC - 1))
                for jn in range(2):
                    dst = bass.AP(tensor=va.tensor,
                                  offset=va.offset + 65 * 8 * jn,
                                  ap=[list(va.ap[0]), [65, 8], [1, 64]])
                    nc.vector.tensor_copy(
                        dst, ps[:, jn * 512:(jn + 1) * 512]
                        .rearrange("p (h d) -> p h d", h=8))
                nc.vector.tensor_scalar_mul(va, va, mask01_sb[:, i:i + 1])

            # -------- AllGather 2: va (overlaps the Q projection) ----
            blobB_loc = dp.tile([TOTB], bf16, name="blobB_loc")
            blobB_g = dp.tile([2 * TOTB], bf16, name="blobB_g")
            nc.sync.dma_start(
                out=blobB_loc.rearrange("(p x) -> p x", p=128),
                in_=val_big[:, :])
            nc.gpsimd.collective_compute(
                "AllGather", mybir.AluOpType.bypass, replica_groups=groups,
                ins=[blobB_loc.opt()], outs=[blobB_g.opt()])

            # ---------------- Q projection + q RMS stats ----------------
            # sumsq matmuls are emitted one (m, j) step late so the PE
            # queue head never blocks on the ACT Square.
            qh_sb = [qhp.tile([128, LC], bf16, name=f"qh{m}", tag="qh")
                     for m in range(KC)]
            sumsq_q = [po.tile([16, 512], fp32, name=f"ssq{j}", tag="pv")
                       for j in range(2)]
            pend = None                      # (m, qsq_tile)
            for m in range(KC):
                ps = pa.tile([128, 1024], fp32, name="proj_ps", tag="pa")
                for k in range(KC):          # k outer: one LDW serves both j
                    for j in range(2):
                        nc.tensor.matmul(
                            ps[:, j * 512:(j + 1) * 512],
                            lhsT=wqc(k)[:, m * 128:(m + 1) * 128],
                            rhs=qtc(k)[:, j * 512:(j + 1) * 512],
                            start=(k == 0), stop=(k == KC - 1))
                if pend is not None:
                    pm, pq = pend
                    for j in range(2):
                        nc.tensor.matmul(
                            sumsq_q[j][:, :], lhsT=indc(pm),
                            rhs=pq[:, j * 512:(j + 1) * 512],
                            start=(pm == 0), stop=(pm == KC - 1))
                nc.vector.tensor_scalar_mul(qh_sb[m][:, :], ps[:, :], qw_sb)
                qsq = kvp.tile([128, 1024], bf16, name="sqt", tag="sq",
                               bufs=6)
                nc.scalar.activation(qsq, ps, AF.Square)
                pend = (m, qsq)
            pm, pq = pend
            for j in range(2):
                nc.tensor.matmul(sumsq_q[j][:, :], lhsT=indc(pm),
                                 rhs=pq[:, j * 512:(j + 1) * 512],
                                 start=False, stop=True)
            # sq = 1/sqrt(mean + eps); bf16 rank-1 broadcast
            sq_sb = []
            for j in range(2):
                sqr = tp.tile([16, 512], fp32, name=f"sqr{j}", tag="small16",
                              bufs=2)
                nc.scalar.activation(sqr, sumsq_q[j][:, :], AF.Sqrt,
                                     scale=1.0 / HD, bias=epsq_sb)
                sqt2 = tp.tile([16, 512], fp32, name=f"sqt2{j}",
                               tag="small16c", bufs=2)
                nc.vector.reciprocal_approx_fast(out=sqt2, in_=sqr)
                sqv = tp.tile([16, 512], bf16, name=f"sqv{j}", tag="small16b",
                              bufs=2)
                nc.vector.tensor_copy(sqv, sqt2)
                sq_sb.append(sqv)
            for m in range(KC):
                for j in range(2):
                    bc = po.tile([128, 512], fp32, name="qbc", tag="pv")
                    nc.tensor.matmul(bc, lhsT=ind2c(m), rhs=sq_sb[j],
                                     start=True, stop=True)
                    nc.vector.tensor_mul(
                        qh_sb[m][:, j * 512:(j + 1) * 512],
                        qh_sb[m][:, j * 512:(j + 1) * 512], bc)

            # -------- readback into canonical full-S tiles --------
            kh_big = khp.tile([128, KC * 2 * W], bf16, name="kh", tag="kh")
            for r in range(2):
                nc.sync.dma_start(
                    out=kh_big.rearrange("p (m rx) -> p m rx", m=KC)
                    [:, :, r * W:(r + 1) * W],
                    in_=blobA_g[r * TOTA:(r + 1) * TOTA]
                    .rearrange("(p m x) -> p m x", p=128, m=KC))
            va_big = vp.tile([128, 2 * n_half * VA_W], bf16, name="vab",
                             tag="vab")
            for r in range(2):
                nc.sync.dma_start(
                    out=va_big[:, r * TOTB // 128:(r + 1) * TOTB // 128],
                    in_=blobB_g[r * TOTB:(r + 1) * TOTB]
                    .rearrange("(p y) -> p y", p=128))

            def khc(m):
                return kh_big[:, m * 2 * W:(m + 1) * 2 * W]

            def vac(i):
                return va_big[:, i * VA_W:(i + 1) * VA_W]

            # wo fetch now: reuses the wk slot (idle since K proj), lands
            # during attention.
            wo_big = wp.tile([128, KC * DIM], bf16, name="wo", tag="wbig")
            nc.sync.dma_start(out=wo_big, in_=wo_d[:, :])

            def woc(k):
                return wo_big[:, k * DIM:(k + 1) * DIM]

            # ------------- attention: one head at a time, ACT-paced -------
            # pv emitted TWO chunks late so it never waits on the exp; the
            # PE keeps a 2-deep score pipeline in the other direction.
            oT_big = qtp.tile([128, KC * LC], bf16, name="oT", tag="qt")

            def oTc(m):
                return oT_big[:, m * LC:(m + 1) * LC]

            den_sb = sp.tile([16, LC], fp32, name="den")
            nc.vector.memset(den_sb, 1.0)    # not-yet-written rows

            def emit_pv(pe):
                ph, pi, pex, ppv, first, last = pe
                for j in range(2):
                    nc.tensor.matmul(
                        ppv[:65, j * 512:(j + 1) * 512],
                        lhsT=vac(pi)[:, ph * 65:(ph + 1) * 65],
                        rhs=pex[:, j * 512:(j + 1) * 512],
                        start=first, stop=last)
                if last:
                    m, r = ph // 2, (ph % 2) * 64
                    dstage = tp.tile([128, LC], fp32, name="dstage",
                                     tag="rec", bufs=2)
                    nc.vector.tensor_copy(dstage[64:65, :], ppv[64:65, :])
                    nc.sync.dma_start(out=den_sb[ph:ph + 1, :],
                                      in_=dstage[64:65, :])
                    nc.vector.tensor_copy(oTc(m)[r:r + 64, :], ppv[0:64, :])

            # force a ramped PE entry into the attention phase: a short
            # continuous burst of junk matmuls (the attention-phase clock
            # is sticky at whatever state it enters with)
            warm0 = po.tile([128, 512], fp32, name="warm0", tag="pv")
            for _ in range(8):
                nc.tensor.matmul(warm0, lhsT=ind2c(0), rhs=ind2_big[:, :512],
                                 start=True, stop=True)
            pending = deque()
            for h in range(H):
                m, r = h // 2, (h % 2) * 64
                pv = po.tile([128, LC], fp32, name=f"pv{h}", tag="pv")
                for i in range(n_sc):
                    sc = pa.tile([128, LC], fp32, name="sc", tag="pa")
                    for j in range(2):
                        nc.tensor.matmul(
                            sc[:, j * 512:(j + 1) * 512],
                            lhsT=khc(m)[r:r + 64, i * 128:(i + 1) * 128],
                            rhs=qh_sb[m][r:r + 64, j * 512:(j + 1) * 512],
                            start=True, stop=True)
                    if len(pending) == 2:
                        emit_pv(pending.popleft())
                    ex = kvp.tile([128, LC], bf16, name="ex", tag="sq",
                                  bufs=6)
                    nc.scalar.activation(ex, sc, AF.Exp)
                    pending.append((h, i, ex, pv, i == 0, i == n_sc - 1))
            while pending:
                emit_pv(pending.popleft())
            # keep the PE clock hot through the last-head drain + recip:
            # a few junk matmuls into a scratch PSUM tile
            warm = pa.tile([128, 512], fp32, name="warm", tag="pa")
            for _ in range(10):
                nc.tensor.matmul(warm, lhsT=ind2c(0), rhs=ind2_big[:, :512],
                                 start=True, stop=True)

            # ------- normalize o^T + output projection, interleaved -------
            denr32 = tp.tile([16, LC], fp32, name="denr32", tag="rec",
                             bufs=2)
            nc.vector.reciprocal_approx_fast(out=denr32, in_=den_sb)
            denr_sb = sp.tile([16, LC], bf16, name="denr")
            nc.vector.tensor_copy(denr_sb, denr32)
            for j in range(2):
                for m in range(KC):
                    obc = po.tile([128, 512], fp32, name="obc", tag="pv")
                    nc.tensor.matmul(obc, lhsT=ind2c(m),
                                     rhs=denr_sb[:, j * 512:(j + 1) * 512],
                                     start=True, stop=True)
                    nc.vector.tensor_mul(
                        oTc(m)[:, j * 512:(j + 1) * 512],
                        oTc(m)[:, j * 512:(j + 1) * 512], obc)
                for lc in range(4 * j, 4 * j + 4):
                    ps = pa.tile([128, 1024], fp32, name="proj_ps",
                                 tag="pa")
                    for k in range(KC):      # k outer: one LDW, both jn
                        for jn in range(2):
                            nc.tensor.matmul(
                                ps[:, jn * 512:(jn + 1) * 512],
                                lhsT=oTc(k)[:, lc * 128:(lc + 1) * 128],
                                rhs=woc(k)[:, jn * 512:(jn + 1) * 512],
                                start=(k == 0), stop=(k == KC - 1))
                    osb = tp.tile([128, 1024], fp32, name="osb", tag="rec",
                                  bufs=2)
                    nc.vector.tensor_copy(osb, ps[:, :])
                    nc.sync.dma_start(
                        out=out_d[lc * 128:(lc + 1) * 128, :], in_=osb)
    nc.compile()
    return nc


def kernel(**inputs):
    q = np.asarray(inputs["q"], dtype=np.float32)
    kv = np.asarray(inputs["kv"], dtype=np.float32)
    seqlens = np.asarray(inputs["x_seqlens"], dtype=np.int32)
    Wq = np.asarray(inputs["Wq"], dtype=np.float32)
    Wk = np.asarray(inputs["Wk"], dtype=np.float32)
    Wv = np.asarray(inputs["Wv"], dtype=np.float32)
    Wo = np.asarray(inputs["Wo"], dtype=np.float32)
    qnw = np.asarray(inputs["q_norm_w"], dtype=np.float32)
    knw = np.asarray(inputs["k_norm_w"], dtype=np.float32)

    n_sc = max(1, int(-(-int(seqlens.max()) // 128)))
    if n_sc not in _CACHE:
        _CACHE[n_sc] = _build(n_sc)
    nc = _CACHE[n_sc]

    def pshuf(w):                 # [KC*128, X] -> [128, KC*X]
        w = np.asarray(w, dtype=np.float32)
        kc, x = w.shape[0] // 128, w.shape[1]
        return np.ascontiguousarray(
            w.reshape(kc, 128, x).transpose(1, 0, 2).reshape(128, kc * x)
        ).astype(BF16)

    wq_b, wk_b = pshuf(Wq), pshuf(Wk)
    wv_b, wo_b = pshuf(Wv), pshuf(Wo)
    qw = np.tile(qnw, 2).reshape(128, 1)
    kw = np.tile(knw, 2).reshape(128, 1)
    ind = np.zeros((KC, 128, 16), np.float32)
    ind2 = np.zeros((KC, 16, 128), np.float32)
    p = np.arange(128)
    for c in range(KC):
        ind[c, p, 2 * c + p // 64] = 1.0
        ind2[c, 2 * c + p // 64, p] = 1.0
    ind = np.ascontiguousarray(
        ind.transpose(1, 0, 2).reshape(128, KC * 16)).astype(BF16)
    ind2 = np.ascontiguousarray(
        ind2.transpose(1, 0, 2).reshape(16, KC * 128)).astype(BF16)

    in_maps = []
    for c in range(N_CORES):
        b, half = c // 2, c % 2
        qT = pshuf(q[b, half * LC:(half + 1) * LC, :].T)
        n_half = (n_sc + 1) // 2
        Wl = n_half * 128
        kvT = pshuf(kv[b].T[:, half * Wl:(half + 1) * Wl])
        sl = int(seqlens[b])
        gpos = half * Wl + np.arange(Wl)          # local kv global positions
        m01 = (gpos < sl).astype(np.float32).reshape(n_half, 128).T
        mask01 = np.zeros((128, 8), np.float32)
        mask01[:, :n_half] = m01
        in_maps.append({
            "qT": qT, "kvT": kvT, "wq": wq_b, "wk": wk_b, "wv": wv_b,
            "wo": wo_b, "mask01": mask01, "qw": qw, "kw": kw, "ind": ind,
            "ind2": ind2,
        })

    res = run_bass_kernel_spmd(nc, in_maps, list(range(N_CORES)),
                               trace=TRACE)
    LAST_RESULT["exec_time_ns"] = res.exec_time_ns
    LAST_RESULT["profile"] = res.profile_json

    out = np.empty((B, L, DIM), np.float32)
    for c in range(N_CORES):
        b, half = c // 2, c % 2
        out[b, half * LC:(half + 1) * LC, :] = res.results[c]["out"]
    return out


# revision 15
# speedup vs baseline: 1.0031x; 1.0031x over previous
"""Trainium2 Bass kernel for CrossAttention (B=4, L=S=2048, DIM=1024, H=16, hd=64).

Sharding: data-parallel over (batch, L-half): core c handles batch c//2,
query rows [(c%2)*1024, (c%2+1)*1024).  Each core computes the QKV
projections for its slice (K/V duplicated within a batch pair), per-head
RMSNorm, masked softmax attention, and the output projection.

Device layout is feature-major ("transposed"): activations live as
[dim, tokens] so every matmul contraction dim is on SBUF partitions with
no on-device transposes.  The host pre-transposes q/kv and casts to bf16.

Softmax: after RMS norm |score| <= 8, so no running max is needed.  The
k-norm rsqrt/8 is folded into kh right after the K projection (rank-2 PE
broadcast via the head-indicator matmul), and the padding mask is folded
into V by zeroing masked va rows (masked exps are finite but contribute
nothing, including to the denominator's ones-column) -- the
attention-phase exp is therefore a BARE activation with no scale/bias AP
reads (~850ns vs ~1400ns per [128,1024] tile).  The denominator comes
from a 65th "ones" column appended to V; the division is deferred: o^T
is stored unnormalized, den rows are collected per head, one fast
approximate reciprocal runs at the end, and a rank-1 bf16 PE broadcast
normalizes o^T interleaved with the output projection per 512-col half.

Perf notes: the Sync engine costs ~650ns per dma_start trigger, so every
multi-tile transfer is batched into ONE dma_start over a big tile (wk,
kvt, wv, wq, qt, wo, ind, ind2 inputs; kh/va gather blobs in
partition-major layout; readbacks).  K-proj operands (wk, kvT) are
issued first so the PE starts ~13us in; wo is fetched during attention
into the recycled wk slot.  The attention loop runs ONE head at a time;
pv matmuls are emitted TWO chunks late so they never wait on the exp:
per 128-pos chunk the PE owes 4 N=512 matmuls (~0.95us) vs one
[128,1024] exp on ACT (~1.1us), so ACT paces the phase.  PSUM exactly
fits: 2x[128,1024] score tiles + 2x[128,1024] pv/aux tiles = 8 banks.
Rank-1 broadcast matmuls (q-norm, k-norm, o-norm) use bf16 operands
(full rate) instead of 4x-slow fp32.
"""

import sys

if "/opt/trn_rl_repo" not in sys.path:
    sys.path.insert(0, "/opt/trn_rl_repo")

from collections import deque

import numpy as np
import ml_dtypes

import concourse.bass as bass
import concourse.bacc as bacc
import concourse.tile as tile
from concourse import mybir
from concourse.bass_utils import run_bass_kernel_spmd

BF16 = ml_dtypes.bfloat16

B, L, S, DIM = 4, 2048, 2048, 1024
H, HD = 16, 64
N_CORES = 8
LC = L // 2          # query rows per core
KC = DIM // 128      # 128-partition chunks of DIM
EPS = 1e-5

TRACE = False        # set by test.py for profiling
LAST_RESULT = {}     # exec_time_ns etc. for test.py

_CACHE = {}


def _build(n_sc):
    """Build the SPMD Bass program; n_sc = number of 128-wide kv chunks."""
    fp32 = mybir.dt.float32
    bf16 = mybir.dt.bfloat16
    AF = mybir.ActivationFunctionType

    nc = bacc.Bacc("TRN2", target_bir_lowering=False, debug=False,
                   num_devices=N_CORES)

    qT_d = nc.dram_tensor("qT", [128, KC * LC], bf16, kind="ExternalInput")
    n_half = (n_sc + 1) // 2         # kv chunks computed locally per core
    W = n_half * 128                 # local kv width
    kvT_d = nc.dram_tensor("kvT", [128, KC * W], bf16, kind="ExternalInput")
    wq_d = nc.dram_tensor("wq", [128, KC * DIM], bf16, kind="ExternalInput")
    wk_d = nc.dram_tensor("wk", [128, KC * DIM], bf16, kind="ExternalInput")
    wv_d = nc.dram_tensor("wv", [128, KC * DIM], bf16, kind="ExternalInput")
    wo_d = nc.dram_tensor("wo", [128, KC * DIM], bf16, kind="ExternalInput")
    mask01_d = nc.dram_tensor("mask01", [128, 8], fp32, kind="ExternalInput")
    qw_d = nc.dram_tensor("qw", [128, 1], fp32, kind="ExternalInput")
    kw_d = nc.dram_tensor("kw", [128, 1], fp32, kind="ExternalInput")
    ind_d = nc.dram_tensor("ind", [128, KC * 16], bf16, kind="ExternalInput")
    ind2_d = nc.dram_tensor("ind2", [16, KC * 128], bf16,
                            kind="ExternalInput")
    out_d = nc.dram_tensor("out", [LC, DIM], fp32, kind="ExternalOutput")

    n_hg = (W + 511) // 512          # 512-wide groups over the local half
    VA_W = H * 65                    # 1040 va columns per chunk

    with tile.TileContext(nc) as tc:
        with (
            tc.tile_pool(name="wp", bufs=3) as wp,               # wk wv wq (+wo)
            tc.tile_pool(name="qtp", bufs=1) as qtp,             # qT, later oT
            tc.tile_pool(name="kvp", bufs=1) as kvp,             # kvT (+sq/exp)
            tc.tile_pool(name="qhp", bufs=KC) as qhp,            # qhT
            tc.tile_pool(name="khp", bufs=1) as khp,             # khT
            tc.tile_pool(name="vp", bufs=1) as vp,               # vh_aug
            tc.tile_pool(name="sp", bufs=1) as sp,               # constants
            tc.tile_pool(name="tp", bufs=4) as tp,               # f32 temps
            tc.tile_pool(name="dp", bufs=1, space="DRAM") as dp,     # blobs
            tc.tile_pool(name="pa", bufs=2, space="PSUM") as pa,     # proj/score
            tc.tile_pool(name="po", bufs=2, space="PSUM") as po,     # pv/aux
        ):
            # ---- inputs, one dma_start each; K-proj operands first ----
            wk_big = wp.tile([128, KC * DIM], bf16, name="wk", tag="wbig")
            nc.sync.dma_start(out=wk_big, in_=wk_d[:, :])
            kvt_big = kvp.tile([128, KC * W], bf16, name="kvt", tag="kv")
            nc.sync.dma_start(out=kvt_big, in_=kvT_d[:, :])
            ind_big = sp.tile([128, KC * 16], bf16, name="ind")
            nc.sync.dma_start(out=ind_big, in_=ind_d[:, :])
            kw_sb = sp.tile([128, 1], fp32, name="kw")
            nc.sync.dma_start(out=kw_sb, in_=kw_d[:, :])
            ind2_big = sp.tile([16, KC * 128], bf16, name="ind2")
            nc.sync.dma_start(out=ind2_big, in_=ind2_d[:, :])
            mask01_sb = sp.tile([128, 8], fp32, name="mask01")
            nc.sync.dma_start(out=mask01_sb, in_=mask01_d[:, :])
            qw_sb = sp.tile([128, 1], fp32, name="qw")
            nc.sync.dma_start(out=qw_sb, in_=qw_d[:, :])
            wv_big = wp.tile([128, KC * DIM], bf16, name="wv", tag="wbig")
            nc.sync.dma_start(out=wv_big, in_=wv_d[:, :])
            wq_big = wp.tile([128, KC * DIM], bf16, name="wq", tag="wbig")
            nc.sync.dma_start(out=wq_big, in_=wq_d[:, :])
            qt_big = qtp.tile([128, KC * LC], bf16, name="qt", tag="qt")
            nc.sync.dma_start(out=qt_big, in_=qT_d[:, :])
            # warmup: pay the one-time CC collective-launch cost (~12us)
            # during the input DMA phase with a tiny dependency-free gather
            wu_in = dp.tile([128], bf16, name="wu_in")
            wu_out = dp.tile([256], bf16, name="wu_out")
            nc.gpsimd.collective_compute(
                "AllGather", mybir.AluOpType.bypass,
                replica_groups=[[2 * x, 2 * x + 1]
                                for x in range(N_CORES // 2)],
                ins=[wu_in.opt()], outs=[wu_out.opt()])

            def wkc(k):
                return wk_big[:, k * DIM:(k + 1) * DIM]

            def wvc(k):
                return wv_big[:, k * DIM:(k + 1) * DIM]

            def wqc(k):
                return wq_big[:, k * DIM:(k + 1) * DIM]

            def qtc(k):
                return qt_big[:, k * LC:(k + 1) * LC]

            def kvtc(k):
                return kvt_big[:, k * W:(k + 1) * W]

            def indc(k):
                return ind_big[:, k * 16:(k + 1) * 16]

            def ind2c(k):
                return ind2_big[:, k * 128:(k + 1) * 128]

            epsq_sb = sp.tile([16, 1], fp32, name="epsq")
            nc.vector.memset(epsq_sb, EPS)
            epsk_sb = sp.tile([16, 1], fp32, name="epsk")
            nc.vector.memset(epsk_sb, 64.0 * EPS)

            # -------- K projection + skT on the LOCAL kv half --------
            khh_big = khp.tile([128, KC * W], bf16, name="khh", tag="khh")
            skrec_sb = [tp.tile([16, 512], bf16, name=f"skrec{sg}",
                                tag="small16b", bufs=2)
                        for sg in range(n_hg)]
            for sg in range(n_hg):
                wdt = min(512, W - sg * 512)
                ssk = po.tile([16, 512], fp32, name="ssk", tag="pv")
                pend = None
                for m in range(KC):
                    ps = pa.tile([128, 1024], fp32, name="proj_ps", tag="pa")
                    for k in range(KC):
                        nc.tensor.matmul(
                            ps[:, :wdt],
                            lhsT=wkc(k)[:, m * 128:(m + 1) * 128],
                            rhs=kvtc(k)[:, sg * 512:sg * 512 + wdt],
                            start=(k == 0), stop=(k == KC - 1))
                    if pend is not None:
                        pm, pq = pend
                        nc.tensor.matmul(
                            ssk[:, :wdt], lhsT=indc(pm), rhs=pq[:, :wdt],
                            start=(pm == 0), stop=False)
                    nc.vector.tensor_scalar_mul(
                        khh_big[:, m * W + sg * 512:m * W + sg * 512 + wdt],
                        ps[:, :wdt], kw_sb)
                    ksq = kvp.tile([128, 1024], bf16, name="sqt", tag="sq",
                                   bufs=6)
                    nc.scalar.activation(ksq[:, :wdt], ps[:, :wdt], AF.Square)
                    pend = (m, ksq)
                pm, pq = pend
                nc.tensor.matmul(ssk[:, :wdt], lhsT=indc(pm),
                                 rhs=pq[:, :wdt], start=False, stop=True)
                # 8*sqrt(mean+eps) = sqrt(sumsq + 64 eps); recip -> sk/8
                skr = tp.tile([16, 512], fp32, name="skr", tag="small16",
                              bufs=2)
                nc.scalar.activation(skr[:, :wdt], ssk[:, :wdt], AF.Sqrt,
                                     scale=1.0, bias=epsk_sb)
                skt = tp.tile([16, 512], fp32, name="skt", tag="small16c",
                              bufs=2)
                nc.vector.reciprocal_approx_fast(out=skt[:, :wdt],
                                                 in_=skr[:, :wdt])
                nc.vector.tensor_copy(skrec_sb[sg][:, :wdt], skt[:, :wdt])

            # -------- scale kh by sk/8, then AllGather 1 (overlaps V) ----
            TOTA = KC * 128 * W
            TOTB = n_half * 128 * VA_W
            groups = [[2 * x, 2 * x + 1] for x in range(N_CORES // 2)]
            blobA_loc = dp.tile([TOTA], bf16, name="blobA_loc")
            blobA_g = dp.tile([2 * TOTA], bf16, name="blobA_g")
            for m in range(KC):
                bcsk = po.tile([128, W], fp32, name="bcsk", tag="pv")
                for sg in range(n_hg):
                    wdt = min(512, W - sg * 512)
                    nc.tensor.matmul(
                        bcsk[:, sg * 512:sg * 512 + wdt],
                        lhsT=ind2c(m), rhs=skrec_sb[sg][:, :wdt],
                        start=True, stop=True)
                nc.vector.tensor_mul(khh_big[:, m * W:(m + 1) * W],
                                     khh_big[:, m * W:(m + 1) * W], bcsk)
            # p-major blob: (p, m, x)
            nc.sync.dma_start(
                out=blobA_loc.rearrange("(p x) -> p x", p=128),
                in_=khh_big[:, :])
            nc.gpsimd.collective_compute(
                "AllGather", mybir.AluOpType.bypass, replica_groups=groups,
                ins=[blobA_loc.opt()], outs=[blobA_g.opt()])

            # -------- V projection on the LOCAL kv half (ones-augmented) ----
            val_big = vp.tile([128, n_half * VA_W], bf16, name="val",
                              tag="val")
            for i in range(n_half):
                va = val_big[:, i * VA_W:(i + 1) * VA_W]
                ones_cols = bass.AP(tensor=va.tensor, offset=va.offset + 64,
                                    ap=[list(va.ap[0]), [65, H], [1, 1]])
                nc.vector.memset(ones_cols, 1.0)
                ps = pa.tile([128, 1024], fp32, name="proj_ps", tag="pa")
                for k in range(KC):          # k outer: one LDW serves both jn
                    for jn in range(2):
                        nc.tensor.matmul(
                            ps[:, jn * 512:(jn + 1) * 512],
                            lhsT=kvtc(k)[:, i * 128:(i + 1) * 128],
                            rhs=wvc(k)[:, jn * 512:(jn + 1) * 512],
                            start=(k == 0), stop=(k == KC - 1))
                for jn in range(2):
                    dst = bass.AP(tensor=va.tensor,
                                  offset=va.offset + 65 * 8 * jn,
                                  ap=[list(va.ap[0]), [65, 8], [1, 64]])
                    nc.vector.tensor_copy(
                        dst, ps[:, jn * 512:(jn + 1) * 512]
                        .rearrange("p (h d) -> p h d", h=8))
                nc.vector.tensor_scalar_mul(va, va, mask01_sb[:, i:i + 1])

            # -------- AllGather 2: va (overlaps the Q projection) ----
            blobB_loc = dp.tile([TOTB], bf16, name="blobB_loc")
            blobB_g = dp.tile([2 * TOTB], bf16, name="blobB_g")
            nc.sync.dma_start(
                out=blobB_loc.rearrange("(p x) -> p x", p=128),
                in_=val_big[:, :])
            nc.gpsimd.collective_compute(
                "AllGather", mybir.AluOpType.bypass, replica_groups=groups,
                ins=[blobB_loc.opt()], outs=[blobB_g.opt()])

            # ---------------- Q projection + q RMS stats ----------------
            # sumsq matmuls are emitted one (m, j) step late so the PE
            # queue head never blocks on the ACT Square.
            qh_sb = [qhp.tile([128, LC], bf16, name=f"qh{m}", tag="qh")
                     for m in range(KC)]
            sumsq_q = [po.tile([16, 512], fp32, name=f"ssq{j}", tag="pv")
                       for j in range(2)]
            pend = None                      # (m, qsq_tile)
            for m in range(KC):
                ps = pa.tile([128, 1024], fp32, name="proj_ps", tag="pa")
                for k in range(KC):          # k outer: one LDW serves both j
                    for j in range(2):
                        nc.tensor.matmul(
                            ps[:, j * 512:(j + 1) * 512],
                            lhsT=wqc(k)[:, m * 128:(m + 1) * 128],
                            rhs=qtc(k)[:, j * 512:(j + 1) * 512],
                            start=(k == 0), stop=(k == KC - 1))
                if pend is not None:
                    pm, pq = pend
                    for j in range(2):
                        nc.tensor.matmul(
                            sumsq_q[j][:, :], lhsT=indc(pm),
                            rhs=pq[:, j * 512:(j + 1) * 512],
                            start=(pm == 0), stop=(pm == KC - 1))
                nc.vector.tensor_scalar_mul(qh_sb[m][:, :], ps[:, :], qw_sb)
                qsq = kvp.tile([128, 1024], bf16, name="sqt", tag="sq",
                               bufs=6)
                nc.scalar.activation(qsq, ps, AF.Square)
                pend = (m, qsq)
            pm, pq = pend
            for j in range(2):
                nc.tensor.matmul(sumsq_q[j][:, :], lhsT=indc(pm),
                                 rhs=pq[:, j * 512:(j + 1) * 512],
                                 start=False, stop=True)
            # sq = 1/sqrt(mean + eps); bf16 rank-1 broadcast
            sq_sb = []
            for j in range(2):
                sqr = tp.tile([16, 512], fp32, name=f"sqr{j}", tag="small16",
                              bufs=2)
                nc.scalar.activation(sqr, sumsq_q[j][:, :], AF.Sqrt,
                                     scale=1.0 / HD, bias=epsq_sb)
                sqt2 = tp.tile([16, 512], fp32, name=f"sqt2{j}",
                               tag="small16c", bufs=2)
                nc.vector.reciprocal_approx_fast(out=sqt2, in_=sqr)
                sqv = tp.tile([16, 512], bf16, name=f"sqv{j}", tag="small16b",
                              bufs=2)
                nc.vector.tensor_copy(sqv, sqt2)
                sq_sb.append(sqv)
            for m in range(KC):
                for j in range(2):
                    bc = po.tile([128, 512], fp32, name="qbc", tag="pv")
                    nc.tensor.matmul(bc, lhsT=ind2c(m), rhs=sq_sb[j],
                                     start=True, stop=True)
                    nc.vector.tensor_mul(
                        qh_sb[m][:, j * 512:(j + 1) * 512],
                        qh_sb[m][:, j * 512:(j + 1) * 512], bc)

            # -------- readback into canonical full-S tiles --------
            kh_big = khp.tile([128, KC * 2 * W], bf16, name="kh", tag="kh")
            for r in range(2):
                nc.sync.dma_start(
                    out=kh_big.rearrange("p (m rx) -> p m rx", m=KC)
                    [:, :, r * W:(r + 1) * W],
                    in_=blobA_g[r * TOTA:(r + 1) * TOTA]
                    .rearrange("(p m x) -> p m x", p=128, m=KC))
            va_big = vp.tile([128, 2 * n_half * VA_W], bf16, name="vab",
                             tag="vab")
            for r in range(2):
                nc.sync.dma_start(
                    out=va_big[:, r * TOTB // 128:(r + 1) * TOTB // 128],
                    in_=blobB_g[r * TOTB:(r + 1) * TOTB]
                    .rearrange("(p y) -> p y", p=128))

            def khc(m):
                return kh_big[:, m * 2 * W:(m + 1) * 2 * W]

            def vac(i):
                return va_big[:, i * VA_W:(i + 1) * VA_W]

            # wo fetch now: reuses the wk slot (idle since K proj), lands
            # during attention.
            wo_big = wp.tile([128, KC * DIM], bf16, name="wo", tag="wbig")
            nc.sync.dma_start(out=wo_big, in_=wo_d[:, :])

            def woc(k):
                return wo_big[:, k * DIM:(k + 1) * DIM]

            # ------------- attention: one head at a time, ACT-paced -------
            # pv emitted TWO chunks late so it never waits on the exp; the
            # PE keeps a 2-deep score pipeline in the other direction.
            oT_big = qtp.tile([128, KC * LC], bf16, name="oT", tag="qt")

            def oTc(m):
                return oT_big[:, m * LC:(m + 1) * LC]

            den_sb = sp.tile([16, LC], fp32, name="den")
            nc.vector.memset(den_sb, 1.0)    # not-yet-written rows

            def emit_pv(pe):
                ph, pi, pex, ppv, first, last = pe
                for j in range(2):
                    nc.tensor.matmul(
                        ppv[:65, j * 512:(j + 1) * 512],
                        lhsT=vac(pi)[:, ph * 65:(ph + 1) * 65],
                        rhs=pex[:, j * 512:(j + 1) * 512],
                        start=first, stop=last)
                if last:
                    m, r = ph // 2, (ph % 2) * 64
                    dstage = tp.tile([128, LC], fp32, name="dstage",
                                     tag="rec", bufs=2)
                    nc.vector.tensor_copy(dstage[64:65, :], ppv[64:65, :])
                    nc.sync.dma_start(out=den_sb[ph:ph + 1, :],
                                      in_=dstage[64:65, :])
                    nc.vector.tensor_copy(oTc(m)[r:r + 64, :], ppv[0:64, :])

            # force a ramped PE entry into the attention phase: a short
            # continuous burst of junk matmuls (the attention-phase clock
            # is sticky at whatever state it enters with)
            warm0 = po.tile([128, 512], fp32, name="warm0", tag="pv")
            for _ in range(8):
                nc.tensor.matmul(warm0, lhsT=ind2c(0), rhs=ind2_big[:, :512],
                                 start=True, stop=True)
            pending = deque()
            for h in range(H):
                m, r = h // 2, (h % 2) * 64
                pv = po.tile([128, LC], fp32, name=f"pv{h}", tag="pv")
                for i in range(n_sc):
                    sc = pa.tile([128, LC], fp32, name="sc", tag="pa")
                    for j in range(2):
                        nc.tensor.matmul(
                            sc[:, j * 512:(j + 1) * 512],
                            lhsT=khc(m)[r:r + 64, i * 128:(i + 1) * 128],
                            rhs=qh_sb[m][r:r + 64, j * 512:(j + 1) * 512],
                            start=True, stop=True)
                    if len(pending) == 2:
                        emit_pv(pending.popleft())
                    ex = kvp.tile([128, LC], bf16, name="ex", tag="sq",
                                  bufs=6)
                    nc.scalar.activation(ex, sc, AF.Exp)
                    pending.append((h, i, ex, pv, i == 0, i == n_sc - 1))
            while pending:
                emit_pv(pending.popleft())
            # keep the PE clock hot through the last-head drain + recip:
            # a few junk matmuls into a scratch PSUM tile
            warm = pa.tile([128, 512], fp32, name="warm", tag="pa")
            for _ in range(10):
                nc.tensor.matmul(warm, lhsT=ind2c(0), rhs=ind2_big[:, :512],
                                 start=True, stop=True)

            # ------- normalize o^T + output projection, interleaved -------
            denr32 = tp.tile([16, LC], fp32, name="denr32", tag="rec",
                             bufs=2)
            nc.vector.reciprocal_approx_fast(out=denr32, in_=den_sb)
            denr_sb = sp.tile([16, LC], bf16, name="denr")
            nc.vector.tensor_copy(denr_sb, denr32)
            for j in range(2):
                for m in range(KC):
                    obc = po.tile([128, 512], fp32, name="obc", tag="pv")
                    nc.tensor.matmul(obc, lhsT=ind2c(m),
                                     rhs=denr_sb[:, j * 512:(j + 1) * 512],
                                     start=True, stop=True)
                    nc.vector.tensor_mul(
                        oTc(m)[:, j * 512:(j + 1) * 512],
                        oTc(m)[:, j * 512:(j + 1) * 512], obc)
                for lc in range(4 * j, 4 * j + 4):
                    ps = pa.tile([128, 1024], fp32, name="proj_ps",
                                 tag="pa")
                    for k in range(KC):      # k outer: one LDW, both jn
                        for jn in range(2):
                            nc.tensor.matmul(
                                ps[:, jn * 512:(jn + 1) * 512],
                                lhsT=oTc(k)[:, lc * 128:(lc + 1) * 128],
                                rhs=woc(k)[:, jn * 512:(jn + 1) * 512],
                                start=(k == 0), stop=(k == KC - 1))
                    osb = tp.tile([128, 1024], fp32, name="osb", tag="rec",
                                  bufs=2)
                    nc.scalar.activation(osb, ps[:, :], AF.Copy)
                    nc.sync.dma_start(
                        out=out_d[lc * 128:(lc + 1) * 128, :], in_=osb)
    nc.compile()
    return nc


def kernel(**inputs):
    q = np.asarray(inputs["q"], dtype=np.float32)
    kv = np.asarray(inputs["kv"], dtype=np.float32)
    seqlens = np.asarray(inputs["x_seqlens"], dtype=np.int32)
    Wq = np.asarray(inputs["Wq"], dtype=np.float32)
    Wk = np.asarray(inputs["Wk"], dtype=np.float32)
    Wv = np.asarray(inputs["Wv"], dtype=np.float32)
    Wo = np.asarray(inputs["Wo"], dtype=np.float32)
    qnw = np.asarray(inputs["q_norm_w"], dtype=np.float32)
    knw = np.asarray(inputs["k_norm_w"], dtype=np.float32)

    n_sc = max(1, int(-(-int(seqlens.max()) // 128)))
    if n_sc not in _CACHE:
        _CACHE[n_sc] = _build(n_sc)
    nc = _CACHE[n_sc]

    def pshuf(w):                 # [KC*128, X] -> [128, KC*X]
        w = np.asarray(w, dtype=np.float32)
        kc, x = w.shape[0] // 128, w.shape[1]
        return np.ascontiguousarray(
            w.reshape(kc, 128, x).transpose(1, 0, 2).reshape(128, kc * x)
        ).astype(BF16)

    wq_b, wk_b = pshuf(Wq), pshuf(Wk)
    wv_b, wo_b = pshuf(Wv), pshuf(Wo)
    qw = np.tile(qnw, 2).reshape(128, 1)
    kw = np.tile(knw, 2).reshape(128, 1)
    ind = np.zeros((KC, 128, 16), np.float32)
    ind2 = np.zeros((KC, 16, 128), np.float32)
    p = np.arange(128)
    for c in range(KC):
        ind[c, p, 2 * c + p // 64] = 1.0
        ind2[c, 2 * c + p // 64, p] = 1.0
    ind = np.ascontiguousarray(
        ind.transpose(1, 0, 2).reshape(128, KC * 16)).astype(BF16)
    ind2 = np.ascontiguousarray(
        ind2.transpose(1, 0, 2).reshape(16, KC * 128)).astype(BF16)

    in_maps = []
    for c in range(N_CORES):
        b, half = c // 2, c % 2
        qT = pshuf(q[b, half * LC:(half + 1) * LC, :].T)
        n_half = (n_sc + 1) // 2
        Wl = n_half * 128
        kvT = pshuf(kv[b].T[:, half * Wl:(half + 1) * Wl])
        sl = int(seqlens[b])
        gpos = half * Wl + np.arange(Wl)          # local kv global positions
        m01 = (gpos < sl).astype(np.float32).reshape(n_half, 128).T
        mask01 = np.zeros((128, 8), np.float32)
        mask01[:, :n_half] = m01
        in_maps.append({
            "qT": qT, "kvT": kvT, "wq": wq_b, "wk": wk_b, "wv": wv_b,
            "wo": wo_b, "mask01": mask01, "qw": qw, "kw": kw, "ind": ind,
            "ind2": ind2,
        })

    res = run_bass_kernel_spmd(nc, in_maps, list(range(N_CORES)),
                               trace=TRACE)
    LAST_RESULT["exec_time_ns"] = res.exec_time_ns
    LAST_RESULT["profile"] = res.profile_json

    out = np.empty((B, L, DIM), np.float32)
    for c in range(N_CORES):
        b, half = c // 2, c % 2
        out[b, half * LC:(half + 1) * LC, :] = res.results[c]["out"]
    return out


# revision 16
# speedup vs baseline: 1.0167x; 1.0136x over previous
"""Trainium2 Bass kernel for CrossAttention (B=4, L=S=2048, DIM=1024, H=16, hd=64).

Sharding: data-parallel over (batch, L-half): core c handles batch c//2,
query rows [(c%2)*1024, (c%2+1)*1024).  Each core computes the QKV
projections for its slice (K/V duplicated within a batch pair), per-head
RMSNorm, masked softmax attention, and the output projection.

Device layout is feature-major ("transposed"): activations live as
[dim, tokens] so every matmul contraction dim is on SBUF partitions with
no on-device transposes.  The host pre-transposes q/kv and casts to bf16.

Softmax: after RMS norm |score| <= 8, so no running max is needed.  The
k-norm rsqrt/8 is folded into kh right after the K projection (rank-2 PE
broadcast via the head-indicator matmul), and the padding mask is folded
into V by zeroing masked va rows (masked exps are finite but contribute
nothing, including to the denominator's ones-column) -- the
attention-phase exp is therefore a BARE activation with no scale/bias AP
reads (~850ns vs ~1400ns per [128,1024] tile).  The denominator comes
from a 65th "ones" column appended to V; the division is deferred: o^T
is stored unnormalized, den rows are collected per head, one fast
approximate reciprocal runs at the end, and a rank-1 bf16 PE broadcast
normalizes o^T interleaved with the output projection per 512-col half.

Perf notes: the Sync engine costs ~650ns per dma_start trigger, so every
multi-tile transfer is batched into ONE dma_start over a big tile (wk,
kvt, wv, wq, qt, wo, ind, ind2 inputs; kh/va gather blobs in
partition-major layout; readbacks).  K-proj operands (wk, kvT) are
issued first so the PE starts ~13us in; wo is fetched during attention
into the recycled wk slot.  The attention loop runs ONE head at a time;
pv matmuls are emitted TWO chunks late so they never wait on the exp:
per 128-pos chunk the PE owes 4 N=512 matmuls (~0.95us) vs one
[128,1024] exp on ACT (~1.1us), so ACT paces the phase.  PSUM exactly
fits: 2x[128,1024] score tiles + 2x[128,1024] pv/aux tiles = 8 banks.
Rank-1 broadcast matmuls (q-norm, k-norm, o-norm) use bf16 operands
(full rate) instead of 4x-slow fp32.
"""

import sys

if "/opt/trn_rl_repo" not in sys.path:
    sys.path.insert(0, "/opt/trn_rl_repo")

from collections import deque

import numpy as np
import ml_dtypes

import concourse.bass as bass
import concourse.bacc as bacc
import concourse.tile as tile
from concourse import mybir
from concourse.bass_utils import run_bass_kernel_spmd

BF16 = ml_dtypes.bfloat16

B, L, S, DIM = 4, 2048, 2048, 1024
H, HD = 16, 64
N_CORES = 8
LC = L // 2          # query rows per core
KC = DIM // 128      # 128-partition chunks of DIM
EPS = 1e-5

TRACE = False        # set by test.py for profiling
LAST_RESULT = {}     # exec_time_ns etc. for test.py

_CACHE = {}


def _build(n_sc):
    """Build the SPMD Bass program; n_sc = number of 128-wide kv chunks."""
    fp32 = mybir.dt.float32
    bf16 = mybir.dt.bfloat16
    AF = mybir.ActivationFunctionType

    nc = bacc.Bacc("TRN2", target_bir_lowering=False, debug=False,
                   num_devices=N_CORES)

    qT_d = nc.dram_tensor("qT", [128, KC * LC], bf16, kind="ExternalInput")
    n_half = (n_sc + 1) // 2         # kv chunks computed locally per core
    W = n_half * 128                 # local kv width
    kvT_d = nc.dram_tensor("kvT", [128, KC * W], bf16, kind="ExternalInput")
    wq_d = nc.dram_tensor("wq", [128, KC * DIM], bf16, kind="ExternalInput")
    wk_d = nc.dram_tensor("wk", [128, KC * DIM], bf16, kind="ExternalInput")
    wv_d = nc.dram_tensor("wv", [128, KC * DIM], bf16, kind="ExternalInput")
    wo_d = nc.dram_tensor("wo", [128, KC * DIM], bf16, kind="ExternalInput")
    mask01_d = nc.dram_tensor("mask01", [128, 8], fp32, kind="ExternalInput")
    qw_d = nc.dram_tensor("qw", [128, 1], fp32, kind="ExternalInput")
    kw_d = nc.dram_tensor("kw", [128, 1], fp32, kind="ExternalInput")
    ind_d = nc.dram_tensor("ind", [128, KC * 16], bf16, kind="ExternalInput")
    ind2_d = nc.dram_tensor("ind2", [16, KC * 128], bf16,
                            kind="ExternalInput")
    out_d = nc.dram_tensor("out", [LC, DIM], fp32, kind="ExternalOutput")

    n_hg = (W + 511) // 512          # 512-wide groups over the local half
    VA_W = H * 65                    # 1040 va columns per chunk

    with tile.TileContext(nc) as tc:
        with (
            tc.tile_pool(name="wp", bufs=3) as wp,               # wk wv wq (+wo)
            tc.tile_pool(name="qtp", bufs=1) as qtp,             # qT, later oT
            tc.tile_pool(name="kvp", bufs=1) as kvp,             # kvT (+sq/exp)
            tc.tile_pool(name="qhp", bufs=KC) as qhp,            # qhT
            tc.tile_pool(name="khp", bufs=1) as khp,             # khT
            tc.tile_pool(name="vp", bufs=1) as vp,               # vh_aug
            tc.tile_pool(name="sp", bufs=1) as sp,               # constants
            tc.tile_pool(name="tp", bufs=4) as tp,               # f32 temps
            tc.tile_pool(name="dp", bufs=1, space="DRAM") as dp,     # blobs
            tc.tile_pool(name="pa", bufs=2, space="PSUM") as pa,     # proj/score
            tc.tile_pool(name="po", bufs=2, space="PSUM") as po,     # pv/aux
        ):
            # ---- inputs, one dma_start each; K-proj operands first ----
            wk_big = wp.tile([128, KC * DIM], bf16, name="wk", tag="wbig")
            nc.sync.dma_start(out=wk_big, in_=wk_d[:, :])
            kvt_big = kvp.tile([128, KC * W], bf16, name="kvt", tag="kv")
            nc.sync.dma_start(out=kvt_big, in_=kvT_d[:, :])
            ind_big = sp.tile([128, KC * 16], bf16, name="ind")
            nc.sync.dma_start(out=ind_big, in_=ind_d[:, :])
            kw_sb = sp.tile([128, 1], fp32, name="kw")
            nc.sync.dma_start(out=kw_sb, in_=kw_d[:, :])
            ind2_big = sp.tile([16, KC * 128], bf16, name="ind2")
            nc.sync.dma_start(out=ind2_big, in_=ind2_d[:, :])
            mask01_sb = sp.tile([128, 8], fp32, name="mask01")
            nc.sync.dma_start(out=mask01_sb, in_=mask01_d[:, :])
            qw_sb = sp.tile([128, 1], fp32, name="qw")
            nc.sync.dma_start(out=qw_sb, in_=qw_d[:, :])
            wv_big = wp.tile([128, KC * DIM], bf16, name="wv", tag="wbig")
            nc.sync.dma_start(out=wv_big, in_=wv_d[:, :])
            wq_big = wp.tile([128, KC * DIM], bf16, name="wq", tag="wbig")
            nc.sync.dma_start(out=wq_big, in_=wq_d[:, :])
            qt_big = qtp.tile([128, KC * LC], bf16, name="qt", tag="qt")
            nc.sync.dma_start(out=qt_big, in_=qT_d[:, :])
            # warmup: pay the one-time CC collective-launch cost (~12us)
            # during the input DMA phase with a tiny dependency-free gather
            wu_in = dp.tile([128], bf16, name="wu_in")
            wu_out = dp.tile([256], bf16, name="wu_out")
            nc.gpsimd.collective_compute(
                "AllGather", mybir.AluOpType.bypass,
                replica_groups=[[2 * x, 2 * x + 1]
                                for x in range(N_CORES // 2)],
                ins=[wu_in.opt()], outs=[wu_out.opt()])

            def wkc(k):
                return wk_big[:, k * DIM:(k + 1) * DIM]

            def wvc(k):
                return wv_big[:, k * DIM:(k + 1) * DIM]

            def wqc(k):
                return wq_big[:, k * DIM:(k + 1) * DIM]

            def qtc(k):
                return qt_big[:, k * LC:(k + 1) * LC]

            def kvtc(k):
                return kvt_big[:, k * W:(k + 1) * W]

            def indc(k):
                return ind_big[:, k * 16:(k + 1) * 16]

            def ind2c(k):
                return ind2_big[:, k * 128:(k + 1) * 128]

            epsq_sb = sp.tile([16, 1], fp32, name="epsq")
            nc.vector.memset(epsq_sb, EPS)
            epsk_sb = sp.tile([16, 1], fp32, name="epsk")
            nc.vector.memset(epsk_sb, 64.0 * EPS)

            # -------- K projection + skT on the LOCAL kv half --------
            khh_big = khp.tile([128, KC * W], bf16, name="khh", tag="khh")
            skrec_sb = [tp.tile([16, 512], bf16, name=f"skrec{sg}",
                                tag="small16b", bufs=2)
                        for sg in range(n_hg)]
            for sg in range(n_hg):
                wdt = min(512, W - sg * 512)
                ssk = po.tile([16, 512], fp32, name="ssk", tag="pv")
                pend = None
                for m in range(KC):
                    ps = pa.tile([128, 1024], fp32, name="proj_ps", tag="pa")
                    for k in range(KC):
                        nc.tensor.matmul(
                            ps[:, :wdt],
                            lhsT=wkc(k)[:, m * 128:(m + 1) * 128],
                            rhs=kvtc(k)[:, sg * 512:sg * 512 + wdt],
                            start=(k == 0), stop=(k == KC - 1))
                    if pend is not None:
                        pm, pq = pend
                        nc.tensor.matmul(
                            ssk[:, :wdt], lhsT=indc(pm), rhs=pq[:, :wdt],
                            start=(pm == 0), stop=False)
                    nc.vector.tensor_scalar_mul(
                        khh_big[:, m * W + sg * 512:m * W + sg * 512 + wdt],
                        ps[:, :wdt], kw_sb)
                    ksq = kvp.tile([128, 1024], bf16, name="sqt", tag="sq",
                                   bufs=6)
                    nc.scalar.activation(ksq[:, :wdt], ps[:, :wdt], AF.Square)
                    pend = (m, ksq)
                pm, pq = pend
                nc.tensor.matmul(ssk[:, :wdt], lhsT=indc(pm),
                                 rhs=pq[:, :wdt], start=False, stop=True)
                # 8*sqrt(mean+eps) = sqrt(sumsq + 64 eps); recip -> sk/8
                skr = tp.tile([16, 512], fp32, name="skr", tag="small16",
                              bufs=2)
                nc.scalar.activation(skr[:, :wdt], ssk[:, :wdt], AF.Sqrt,
                                     scale=1.0, bias=epsk_sb)
                skt = tp.tile([16, 512], fp32, name="skt", tag="small16c",
                              bufs=2)
                nc.vector.reciprocal_approx_fast(out=skt[:, :wdt],
                                                 in_=skr[:, :wdt])
                nc.vector.tensor_copy(skrec_sb[sg][:, :wdt], skt[:, :wdt])

            # -------- scale kh by sk/8, then AllGather 1 (overlaps V) ----
            TOTA = KC * 128 * W
            TOTB = n_half * 128 * VA_W
            groups = [[2 * x, 2 * x + 1] for x in range(N_CORES // 2)]
            blobA_loc = dp.tile([TOTA], bf16, name="blobA_loc")
            blobA_g = dp.tile([2 * TOTA], bf16, name="blobA_g")
            for m in range(KC):
                bcsk = po.tile([128, W], fp32, name="bcsk", tag="pv")
                for sg in range(n_hg):
                    wdt = min(512, W - sg * 512)
                    nc.tensor.matmul(
                        bcsk[:, sg * 512:sg * 512 + wdt],
                        lhsT=ind2c(m), rhs=skrec_sb[sg][:, :wdt],
                        start=True, stop=True)
                nc.vector.tensor_mul(khh_big[:, m * W:(m + 1) * W],
                                     khh_big[:, m * W:(m + 1) * W], bcsk)
            # p-major blob: (p, m, x)
            nc.sync.dma_start(
                out=blobA_loc.rearrange("(p x) -> p x", p=128),
                in_=khh_big[:, :])
            nc.gpsimd.collective_compute(
                "AllGather", mybir.AluOpType.bypass, replica_groups=groups,
                ins=[blobA_loc.opt()], outs=[blobA_g.opt()])

            # -------- V projection on the LOCAL kv half (ones-augmented) ----
            val_big = vp.tile([128, n_half * VA_W], bf16, name="val",
                              tag="val")
            for i in range(n_half):
                va = val_big[:, i * VA_W:(i + 1) * VA_W]
                ones_cols = bass.AP(tensor=va.tensor, offset=va.offset + 64,
                                    ap=[list(va.ap[0]), [65, H], [1, 1]])
                nc.vector.memset(ones_cols, 1.0)
                ps = pa.tile([128, 1024], fp32, name="proj_ps", tag="pa")
                for k in range(KC):          # k outer: one LDW serves both jn
                    for jn in range(2):
                        nc.tensor.matmul(
                            ps[:, jn * 512:(jn + 1) * 512],
                            lhsT=kvtc(k)[:, i * 128:(i + 1) * 128],
                            rhs=wvc(k)[:, jn * 512:(jn + 1) * 512],
                            start=(k == 0), stop=(k == KC - 1))
                for jn in range(2):
                    dst = bass.AP(tensor=va.tensor,
                                  offset=va.offset + 65 * 8 * jn,
                                  ap=[list(va.ap[0]), [65, 8], [1, 64]])
                    nc.vector.tensor_copy(
                        dst, ps[:, jn * 512:(jn + 1) * 512]
                        .rearrange("p (h d) -> p h d", h=8))
                nc.vector.tensor_scalar_mul(va, va, mask01_sb[:, i:i + 1])

            # -------- AllGather 2: va (overlaps the Q projection) ----
            blobB_loc = dp.tile([TOTB], bf16, name="blobB_loc")
            blobB_g = dp.tile([2 * TOTB], bf16, name="blobB_g")
            nc.sync.dma_start(
                out=blobB_loc.rearrange("(p x) -> p x", p=128),
                in_=val_big[:, :])
            nc.gpsimd.collective_compute(
                "AllGather", mybir.AluOpType.bypass, replica_groups=groups,
                ins=[blobB_loc.opt()], outs=[blobB_g.opt()])

            # ---------------- Q projection + q RMS stats ----------------
            # sumsq matmuls are emitted one (m, j) step late so the PE
            # queue head never blocks on the ACT Square.
            qh_sb = [qhp.tile([128, LC], bf16, name=f"qh{m}", tag="qh")
                     for m in range(KC)]
            sumsq_q = [po.tile([16, 512], fp32, name=f"ssq{j}", tag="pv")
                       for j in range(2)]
            pend = None                      # (m, qsq_tile)
            for m in range(KC):
                ps = pa.tile([128, 1024], fp32, name="proj_ps", tag="pa")
                for k in range(KC):          # k outer: one LDW serves both j
                    for j in range(2):
                        nc.tensor.matmul(
                            ps[:, j * 512:(j + 1) * 512],
                            lhsT=wqc(k)[:, m * 128:(m + 1) * 128],
                            rhs=qtc(k)[:, j * 512:(j + 1) * 512],
                            start=(k == 0), stop=(k == KC - 1))
                if pend is not None:
                    pm, pq = pend
                    for j in range(2):
                        nc.tensor.matmul(
                            sumsq_q[j][:, :], lhsT=indc(pm),
                            rhs=pq[:, j * 512:(j + 1) * 512],
                            start=(pm == 0), stop=(pm == KC - 1))
                nc.vector.tensor_scalar_mul(qh_sb[m][:, :], ps[:, :], qw_sb)
                qsq = kvp.tile([128, 1024], bf16, name="sqt", tag="sq",
                               bufs=6)
                nc.scalar.activation(qsq, ps, AF.Square)
                pend = (m, qsq)
            pm, pq = pend
            for j in range(2):
                nc.tensor.matmul(sumsq_q[j][:, :], lhsT=indc(pm),
                                 rhs=pq[:, j * 512:(j + 1) * 512],
                                 start=False, stop=True)
            # sq = 1/sqrt(mean + eps); bf16 rank-1 broadcast
            sq_sb = []
            for j in range(2):
                sqr = tp.tile([16, 512], fp32, name=f"sqr{j}", tag="small16",
                              bufs=2)
                nc.scalar.activation(sqr, sumsq_q[j][:, :], AF.Sqrt,
                                     scale=1.0 / HD, bias=epsq_sb)
                sqt2 = tp.tile([16, 512], fp32, name=f"sqt2{j}",
                               tag="small16c", bufs=2)
                nc.vector.reciprocal_approx_fast(out=sqt2, in_=sqr)
                sqv = tp.tile([16, 512], bf16, name=f"sqv{j}", tag="small16b",
                              bufs=2)
                nc.vector.tensor_copy(sqv, sqt2)
                sq_sb.append(sqv)
            for m in range(KC):
                for j in range(2):
                    bc = po.tile([128, 512], fp32, name="qbc", tag="pv")
                    nc.tensor.matmul(bc, lhsT=ind2c(m), rhs=sq_sb[j],
                                     start=True, stop=True)
                    nc.vector.tensor_mul(
                        qh_sb[m][:, j * 512:(j + 1) * 512],
                        qh_sb[m][:, j * 512:(j + 1) * 512], bc)

            # -------- readback into canonical full-S tiles --------
            kh_big = khp.tile([128, KC * 2 * W], bf16, name="kh", tag="kh")
            for r in range(2):
                nc.sync.dma_start(
                    out=kh_big.rearrange("p (m rx) -> p m rx", m=KC)
                    [:, :, r * W:(r + 1) * W],
                    in_=blobA_g[r * TOTA:(r + 1) * TOTA]
                    .rearrange("(p m x) -> p m x", p=128, m=KC))
            va_big = vp.tile([128, 2 * n_half * VA_W], bf16, name="vab",
                             tag="vab")
            for r in range(2):
                nc.sync.dma_start(
                    out=va_big[:, r * TOTB // 128:(r + 1) * TOTB // 128],
                    in_=blobB_g[r * TOTB:(r + 1) * TOTB]
                    .rearrange("(p y) -> p y", p=128))

            def khc(m):
                return kh_big[:, m * 2 * W:(m + 1) * 2 * W]

            def vac(i):
                return va_big[:, i * VA_W:(i + 1) * VA_W]

            # wo fetch now: reuses the wk slot (idle since K proj), lands
            # during attention.
            wo_big = wp.tile([128, KC * DIM], bf16, name="wo", tag="wbig")
            nc.sync.dma_start(out=wo_big, in_=wo_d[:, :])

            def woc(k):
                return wo_big[:, k * DIM:(k + 1) * DIM]

            # ------------- attention: one head at a time, ACT-paced -------
            # pv emitted TWO chunks late so it never waits on the exp; the
            # PE keeps a 2-deep score pipeline in the other direction.
            oT_big = qtp.tile([128, KC * LC], bf16, name="oT", tag="qt")

            def oTc(m):
                return oT_big[:, m * LC:(m + 1) * LC]

            den_sb = sp.tile([16, LC], fp32, name="den")
            nc.vector.memset(den_sb, 1.0)    # not-yet-written rows

            def emit_pv(pe):
                ph, pi, pex, ppv, first, last = pe
                for j in range(2):
                    nc.tensor.matmul(
                        ppv[:65, j * 512:(j + 1) * 512],
                        lhsT=vac(pi)[:, ph * 65:(ph + 1) * 65],
                        rhs=pex[:, j * 512:(j + 1) * 512],
                        start=first, stop=last)
                if last:
                    m, r = ph // 2, (ph % 2) * 64
                    dstage = tp.tile([128, LC], fp32, name="dstage",
                                     tag="rec", bufs=2)
                    nc.vector.tensor_copy(dstage[64:65, :], ppv[64:65, :])
                    nc.sync.dma_start(out=den_sb[ph:ph + 1, :],
                                      in_=dstage[64:65, :])
                    nc.vector.tensor_copy(oTc(m)[r:r + 64, :], ppv[0:64, :])

            # force a ramped PE entry into the attention phase: a short
            # continuous burst of junk matmuls (the attention-phase clock
            # is sticky at whatever state it enters with)
            warm0 = pa.tile([128, 512], fp32, name="warm0", tag="pa")
            for _ in range(8):
                nc.tensor.matmul(warm0, lhsT=ind2c(0), rhs=ind2_big[:, :512],
                                 start=True, stop=True)
            pending = deque()
            for h in range(H):
                m, r = h // 2, (h % 2) * 64
                pv = po.tile([128, LC], fp32, name=f"pv{h}", tag="pv")
                for i in range(n_sc):
                    sc = pa.tile([128, LC], fp32, name="sc", tag="pa")
                    for j in range(2):
                        nc.tensor.matmul(
                            sc[:, j * 512:(j + 1) * 512],
                            lhsT=khc(m)[r:r + 64, i * 128:(i + 1) * 128],
                            rhs=qh_sb[m][r:r + 64, j * 512:(j + 1) * 512],
                            start=True, stop=True)
                    if len(pending) == 2:
                        emit_pv(pending.popleft())
                    ex = kvp.tile([128, LC], bf16, name="ex", tag="sq",
                                  bufs=6)
                    nc.scalar.activation(ex, sc, AF.Exp)
                    pending.append((h, i, ex, pv, i == 0, i == n_sc - 1))
            while pending:
                emit_pv(pending.popleft())
            # keep the PE clock hot through the last-head drain + recip:
            # a few junk matmuls into a scratch PSUM tile
            warm = pa.tile([128, 512], fp32, name="warm", tag="pa")
            for _ in range(10):
                nc.tensor.matmul(warm, lhsT=ind2c(0), rhs=ind2_big[:, :512],
                                 start=True, stop=True)

            # ------- normalize o^T + output projection, interleaved -------
            denr32 = tp.tile([16, LC], fp32, name="denr32", tag="rec",
                             bufs=2)
            nc.vector.reciprocal_approx_fast(out=denr32, in_=den_sb)
            denr_sb = sp.tile([16, LC], bf16, name="denr")
            nc.vector.tensor_copy(denr_sb, denr32)
            for j in range(2):
                for m in range(KC):
                    obc = po.tile([128, 512], fp32, name="obc", tag="pv")
                    nc.tensor.matmul(obc, lhsT=ind2c(m),
                                     rhs=denr_sb[:, j * 512:(j + 1) * 512],
                                     start=True, stop=True)
                    nc.vector.tensor_mul(
                        oTc(m)[:, j * 512:(j + 1) * 512],
                        oTc(m)[:, j * 512:(j + 1) * 512], obc)
                for lc in range(4 * j, 4 * j + 4):
                    ps = pa.tile([128, 1024], fp32, name="proj_ps",
                                 tag="pa")
                    for k in range(KC):      # k outer: one LDW, both jn
                        for jn in range(2):
                            nc.tensor.matmul(
                                ps[:, jn * 512:(jn + 1) * 512],
                                lhsT=oTc(k)[:, lc * 128:(lc + 1) * 128],
                                rhs=woc(k)[:, jn * 512:(jn + 1) * 512],
                                start=(k == 0), stop=(k == KC - 1))
                    osb = tp.tile([128, 1024], fp32, name="osb", tag="rec",
                                  bufs=2)
                    nc.scalar.activation(osb, ps[:, :], AF.Copy)
                    nc.sync.dma_start(
                        out=out_d[lc * 128:(lc + 1) * 128, :], in_=osb)
    nc.compile()
    return nc


def kernel(**inputs):
    q = np.asarray(inputs["q"], dtype=np.float32)
    kv = np.asarray(inputs["kv"], dtype=np.float32)
    seqlens = np.asarray(inputs["x_seqlens"], dtype=np.int32)
    Wq = np.asarray(inputs["Wq"], dtype=np.float32)
    Wk = np.asarray(inputs["Wk"], dtype=np.float32)
    Wv = np.asarray(inputs["Wv"], dtype=np.float32)
    Wo = np.asarray(inputs["Wo"], dtype=np.float32)
    qnw = np.asarray(inputs["q_norm_w"], dtype=np.float32)
    knw = np.asarray(inputs["k_norm_w"], dtype=np.float32)

    n_sc = max(1, int(-(-int(seqlens.max()) // 128)))
    if n_sc not in _CACHE:
        _CACHE[n_sc] = _build(n_sc)
    nc = _CACHE[n_sc]

    def pshuf(w):                 # [KC*128, X] -> [128, KC*X]
        w = np.asarray(w, dtype=np.float32)
        kc, x = w.shape[0] // 128, w.shape[1]
        return np.ascontiguousarray(
            w.reshape(kc, 128, x).transpose(1, 0, 2).reshape(128, kc * x)
        ).astype(BF16)

    wq_b, wk_b = pshuf(Wq), pshuf(Wk)
    wv_b, wo_b = pshuf(Wv), pshuf(Wo)
    qw = np.tile(qnw, 2).reshape(128, 1)
    kw = np.tile(knw, 2).reshape(128, 1)
    ind = np.zeros((KC, 128, 16), np.float32)
    ind2 = np.zeros((KC, 16, 128), np.float32)
    p = np.arange(128)
    for c in range(KC):
        ind[c, p, 2 * c + p // 64] = 1.0
        ind2[c, 2 * c + p // 64, p] = 1.0
    ind = np.ascontiguousarray(
        ind.transpose(1, 0, 2).reshape(128, KC * 16)).astype(BF16)
    ind2 = np.ascontiguousarray(
        ind2.transpose(1, 0, 2).reshape(16, KC * 128)).astype(BF16)

    in_maps = []
    for c in range(N_CORES):
        b, half = c // 2, c % 2
        qT = pshuf(q[b, half * LC:(half + 1) * LC, :].T)
        n_half = (n_sc + 1) // 2
        Wl = n_half * 128
        kvT = pshuf(kv[b].T[:, half * Wl:(half + 1) * Wl])
        sl = int(seqlens[b])
        gpos = half * Wl + np.arange(Wl)          # local kv global positions
        m01 = (gpos < sl).astype(np.float32).reshape(n_half, 128).T
        mask01 = np.zeros((128, 8), np.float32)
        mask01[:, :n_half] = m01
        in_maps.append({
            "qT": qT, "kvT": kvT, "wq": wq_b, "wk": wk_b, "wv": wv_b,
            "wo": wo_b, "mask01": mask01, "qw": qw, "kw": kw, "ind": ind,
            "ind2": ind2,
        })

    res = run_bass_kernel_spmd(nc, in_maps, list(range(N_CORES)),
                               trace=TRACE)
    LAST_RESULT["exec_time_ns"] = res.exec_time_ns
    LAST_RESULT["profile"] = res.profile_json

    out = np.empty((B, L, DIM), np.float32)
    for c in range(N_CORES):
        b, half = c // 2, c % 2
        out[b, half * LC:(half + 1) * LC, :] = res.results[c]["out"]
    return out
